# revision 1
# baseline (speedup 1.0000x reference)
"""Trainium2 Bass kernel for nn_LogBessel: out = log(I_31(kappa) + 1e-10).

Math: instead of the reference's 128-term log-space power series, use the
exact identity (uniform asymptotic / Debye structure)

    ln I_nu(x) = W - nu*ln(nu + W) + nu*ln(x) + P(y),
    W = sqrt(nu^2 + x^2),  y = ln(W^2),  nu = 31

where P(y) = -0.25*y - 0.5*ln(2*pi) + ln(sum_k u_k/nu^k) is smooth and tiny;
it is fitted offline as a degree-4 polynomial in y over y in [ln 961,
ln 3461] (max fit error 6.7e-7, fp32 Horner error 1.4e-6 -- both far below
the fp32 noise of the reference itself).

Engine split per [128 x 2048] chunk:
  ScalarE (ACT, one natural_log_exp table set, no table switching):
      L = Ln(x); y = Ln(x^2+961); W = Exp(0.5*y); q = Ln(W+31);
      iv = Exp(g); out = Ln(iv + 1e-10)
  (GpSimd stays idle: it shares SBUF ports with VectorE, so offloading
   elementwise work there slows VectorE down.)
  VectorE: Horner for P(y) + assembly, via fused scalar_tensor_tensor
           (out = (in0 op0 scalar) op1 in1).

The final Exp/Ln pair reproduces the reference's exp(log_iv) + eps -> log
structure, so the small-x regime (output == log(1e-10)) matches exactly.

Sharding: trivially data-parallel; 4096 rows split into 8 blocks of 512,
one per NeuronCore (same SPMD program, different data).
"""

import numpy as np

from concourse import bacc, mybir, tile
from concourse import bass_utils

F32 = mybir.dt.float32
AF = mybir.ActivationFunctionType
OP = mybir.AluOpType

N_CORES = 8
ROWS, COLS = 4096, 4096
SH_ROWS = ROWS // N_CORES          # 512 rows per core
P = 128                            # SBUF partitions
FD = 2048                          # free-dim chunk size
ROW_BLOCKS = SH_ROWS // P          # 4
COL_BLOCKS = COLS // FD            # 2

# deg-4 fit of P(y) on [ln 961, ln 3461], power basis (see docstring)
A0 = -3.087667582403775
A1 = 0.7840119052482061
A2 = -0.18577208264273426
A3 = 0.014913698452924522
A4 = -0.00045134658423458393
EPS = 1e-10

_nc_cache = None



_ACT_SET = "natural_log_exp_and_others"


def _force_single_act_set():
    """Make ln/exp/square resolvable only from natural_log_exp_and_others so
    walrus's per-function set assignment cannot ping-pong table loads."""
    import json, tempfile, os
    try:
        from neuronxcc.driver.jobs.support import FindActInfo
        from neuronxcc.driver.jobs import WalrusDriver as WD
    except ImportError:
        return
    if getattr(FindActInfo, "_logbessel_patched", False):
        return
    orig = FindActInfo.findActInfoFile

    def patched(package_dir, arch):
        path = orig(package_dir, arch)
        try:
            import shutil
            # table .bin blobs are resolved relative to the json, so clone
            # the whole pwp_bin dir and patch the json inside the clone
            dst = os.path.join(tempfile.gettempdir(), "pwp_single_set")
            if not os.path.isdir(dst):
                shutil.copytree(os.path.dirname(path), dst)
            d = json.load(open(path))
            for s in d.get("act_func_sets", []):
                if s.get("name") != _ACT_SET:
                    for fn in ("ln", "exp", "square"):
                        s.get("act", {}).pop(fn, None)
            out = os.path.join(dst, "act_info.json")
            with open(out, "w") as f:
                json.dump(d, f)
            return out
        except Exception:
            return path

    patched._logbessel_patched = True
    FindActInfo._logbessel_patched = True
    FindActInfo.findActInfoFile = patched
    WD.findActInfoFile = patched


def _build():
    _force_single_act_set()
    nc = bacc.Bacc("TRN2", target_bir_lowering=False, debug=False)
    x = nc.dram_tensor("x", [SH_ROWS, COLS], F32, kind="ExternalInput").ap()
    y = nc.dram_tensor("y", [SH_ROWS, COLS], F32, kind="ExternalOutput").ap()

    # activation() requires float biases to exist as [128,1] const SBUF
    # tensors; register ours the same way Bass.__init__ registers 0.0/1.0.
    for val in (961.0, 31.0, EPS, A0):
        t = nc.alloc_sbuf_tensor(f"const-f32-{val}", [128, 1], F32)
        nc.gpsimd.memset(t.ap(), val)
        nc.const_aps.aps[(F32, val)] = t.ap()
    nc.all_engine_barrier()

    with tile.TileContext(nc) as tc:
        with tc.tile_pool(name="p", bufs=2) as pool:
            for c in range(ROW_BLOCKS):
                for d in range(COL_BLOCKS):
                    rs = slice(c * P, (c + 1) * P)
                    cs = slice(d * FD, (d + 1) * FD)

                    tx = pool.tile([P, FD], F32, tag="x")
                    nc.sync.dma_start(tx[:], x[rs, cs])

                    tx2 = pool.tile([P, FD], F32, tag="x2")
                    nc.scalar.activation(tx2[:], tx[:], AF.Square)
                    tL = pool.tile([P, FD], F32, tag="L")
                    nc.scalar.activation(tL[:], tx[:], AF.Ln)
                    ty = pool.tile([P, FD], F32, tag="y")
                    nc.scalar.activation(ty[:], tx2[:], AF.Ln, bias=961.0)
                    tW = pool.tile([P, FD], F32, tag="W")
                    nc.scalar.activation(tW[:], ty[:], AF.Exp, scale=0.5)
                    tq = pool.tile([P, FD], F32, tag="q")
                    nc.scalar.activation(tq[:], tW[:], AF.Ln, bias=31.0)

                    # Horner for P(y): H = a4*y; H = (H + a_j)*y
                    tG = pool.tile([P, FD], F32, tag="G")
                    nc.vector.tensor_scalar_mul(tG[:], ty[:], A4)
                    nc.vector.scalar_tensor_tensor(
                        tG[:], tG[:], A3, ty[:], op0=OP.add, op1=OP.mult)
                    nc.vector.scalar_tensor_tensor(
                        tG[:], tG[:], A2, ty[:], op0=OP.add, op1=OP.mult)
                    nc.vector.scalar_tensor_tensor(
                        tG[:], tG[:], A1, ty[:], op0=OP.add, op1=OP.mult)

                    # assembly: g = W - 31*ln(31+W) + 31*ln(x) + H + a0
                    ts_ = pool.tile([P, FD], F32, tag="s")
                    nc.vector.scalar_tensor_tensor(
                        ts_[:], tq[:], -31.0, tW[:], op0=OP.mult, op1=OP.add)
                    nc.vector.scalar_tensor_tensor(
                        ts_[:], tL[:], 31.0, ts_[:], op0=OP.mult, op1=OP.add)
                    tg = pool.tile([P, FD], F32, tag="g")
                    nc.vector.tensor_tensor(tg[:], tG[:], ts_[:], OP.add)

                    # out = ln(exp(g + a0) + eps)  (a0 folded into Exp bias;
                    # same exp -> +eps -> log structure as the reference)
                    to = pool.tile([P, FD], F32, tag="o")
                    nc.scalar.activation(to[:], tg[:], AF.Exp, bias=A0)
                    nc.scalar.activation(to[:], to[:], AF.Ln, bias=EPS)

                    nc.sync.dma_start(y[rs, cs], to[:])

    nc.compile()
    return nc


def _get_nc():
    global _nc_cache
    if _nc_cache is None:
        _nc_cache = _build()
    return _nc_cache


def kernel(kappa: np.ndarray) -> np.ndarray:
    kappa = np.ascontiguousarray(np.asarray(kappa, dtype=np.float32))
    assert kappa.shape == (ROWS, COLS)
    nc = _get_nc()
    in_maps = [
        {"x": kappa[i * SH_ROWS:(i + 1) * SH_ROWS]} for i in range(N_CORES)
    ]
    res = bass_utils.run_bass_kernel_spmd(
        nc, in_maps, core_ids=list(range(N_CORES)))
    out = np.concatenate([res.results[i]["y"] for i in range(N_CORES)], axis=0)
    return out.astype(np.float32)



# revision 3
# speedup vs baseline: 2.8539x; 2.8539x over previous
"""Trainium2 Bass kernel for nn_LogBessel: out = log(I_31(kappa) + 1e-10).

Math: the reference's f(x) = ln(exp(ln I_31(x)) + eps) is approximated as

    f(x) ~= ln(exp(s*M(t) + b) + eps),   t = ALPHA * x  (host-side affine)
    M(t) = (((t + c0)*t + c1)*t + c2)*t  (monic degree-4 Horner, no const term)

The Exp/Ln pair reproduces the reference's exp -> +eps -> log structure
exactly, so the small-x regime (output == log(1e-10)) matches by
construction, and the deg-4 polynomial only needs accuracy where
exp(g) >~ eps (x in ~[11, 50]).  Offline end-to-end minimax fit gives
max |f_hat - f| ~= 0.066 (fp64) / ~0.1 with fp16 I/O quantization --
~6x under the 2e-2-relative harness gate (abs budget ~0.75).

Engine split per [128 x 2048] chunk:
  DVE:     ONE custom-DVE op (6 ALU stages) evaluates M(t) in a single
           streaming pass (fp16 in, fp32 out).  Stock-op alternatives
           (scalar_tensor_tensor chains) run 1x and need 4+ passes.
  ScalarE: Exp(s*M + b) then Ln(iv + eps)  (both in the
           natural_log_exp_and_others table set -- no table reloads).
  DMA:     fp16 both directions (host converts fp32->fp16 with the
           ALPHA prescale folded in; output upcast fp16->fp32 on host).
           Per-core traffic 2 x 4 MB ~= 25 us at ~330 GB/s -- the
           roofline this kernel targets.

Sharding: trivially data-parallel; 4096 rows split into 8 blocks of 512,
one per NeuronCore (same SPMD program, different data).
"""

import numpy as np

from concourse import bacc, mybir, tile
from concourse import bass_utils

F16 = mybir.dt.float16
F32 = mybir.dt.float32
AF = mybir.ActivationFunctionType

N_CORES = 8
ROWS, COLS = 4096, 4096
SH_ROWS = ROWS // N_CORES          # 512 rows per core
P = 128                            # SBUF partitions
FD = 2048                          # free-dim chunk size
ROW_BLOCKS = SH_ROWS // P          # 4
COL_BLOCKS = COLS // FD            # 2

# --- fitted constants (see module docstring; offline minimax fit) ---
ALPHA = 0.259912064                # host prescale: t = ALPHA * kappa
PC0 = -40.4114600                  # M = (((t+PC0)*t+PC1)*t+PC2)*t
PC1 = 636.127312
PC2 = -6194.10922
PS = -3.11125819e-3                # iv = exp(PS*M + PBEXP)
PBEXP = -65.7643952                # = b + ln(eps)
EPS = 1e-10

_nc_cache = None

_ACT_SET = "natural_log_exp_and_others"


def _force_single_act_set():
    """Make ln/exp resolvable only from natural_log_exp_and_others so
    walrus's per-function set assignment cannot ping-pong table loads."""
    import json, tempfile, os
    try:
        from neuronxcc.driver.jobs.support import FindActInfo
        from neuronxcc.driver.jobs import WalrusDriver as WD
    except ImportError:
        return
    if getattr(FindActInfo, "_logbessel_patched", False):
        return
    orig = FindActInfo.findActInfoFile

    def patched(package_dir, arch):
        path = orig(package_dir, arch)
        try:
            import shutil
            # table .bin blobs are resolved relative to the json, so clone
            # the whole pwp_bin dir and patch the json inside the clone
            dst = os.path.join(tempfile.gettempdir(), "pwp_single_set")
            if not os.path.isdir(dst):
                shutil.copytree(os.path.dirname(path), dst)
            d = json.load(open(path))
            for s in d.get("act_func_sets", []):
                if s.get("name") != _ACT_SET:
                    for fn in ("ln", "exp", "square"):
                        s.get("act", {}).pop(fn, None)
            out = os.path.join(dst, "act_info.json")
            with open(out, "w") as f:
                json.dump(d, f)
            return out
        except Exception:
            return path

    patched._logbessel_patched = True
    FindActInfo._logbessel_patched = True
    FindActInfo.findActInfoFile = patched
    WD.findActInfoFile = patched


_OP_NAME = "LOGBESSEL_P4_ANT"


def _register_custom_op():
    """Register the monic deg-4 Horner as a custom DVE op:
    out = (((in0 + s0)*in0 + s1)*in0 + imm2)*in0   -- 6 ALU stages, one
    streaming DVE pass per tile."""
    import concourse.dve_ops as dve_ops_mod
    from concourse.dve_ops import DveOp
    from concourse.dve_spec import Spec, Src0, C0, C1, C2, lower as dve_lower
    from concourse.dve_uop import DveOpSpec

    for op in dve_ops_mod.OPS:
        if op.name == _OP_NAME:
            return op

    body = (((Src0 + C0) * Src0 + C1) * Src0 + C2) * Src0
    spec = Spec(
        body=body,
        reference=lambda in0, in1, s0, s1, imm2: (
            (((in0.astype(np.float32) + s0) * in0 + s1) * in0 + imm2) * in0
        ).astype(np.float32),
    )
    row = max(dve_ops_mod._SUB_OPCODE_FOR_NAME.values()) + 1
    assert row < 0x20, "custom-DVE 5-bit row space exhausted"
    dve_ops_mod._SUB_OPCODE_FOR_NAME[_OP_NAME] = row
    shas = {}
    for ver in ("v3", "v4"):
        uops = dve_lower(spec, ver=ver)
        shas[ver] = DveOpSpec(
            name=_OP_NAME, opcode=row, uops=uops, rd1_en=False
        ).sha(ver)
    op = DveOp(_OP_NAME, spec, subdim=False, uops_sha=shas)
    dve_ops_mod.OPS.append(op)
    dve_ops_mod.CUSTOM_DVE_SPECS[_OP_NAME] = spec
    return op


def _build():
    _force_single_act_set()
    poly_op = _register_custom_op()

    nc = bacc.Bacc("TRN2", target_bir_lowering=False, debug=False)
    x = nc.dram_tensor("x", [SH_ROWS, COLS], F16, kind="ExternalInput").ap()
    y = nc.dram_tensor("y", [SH_ROWS, COLS], F16, kind="ExternalOutput").ap()

    # activation() requires float biases to exist as [128,1] const SBUF
    # tensors; register ours the same way Bass.__init__ registers 0.0/1.0.
    for val in (PBEXP, EPS):
        t = nc.alloc_sbuf_tensor(f"const-f32-{val}", [128, 1], F32)
        nc.gpsimd.memset(t.ap(), val)
        nc.const_aps.aps[(F32, val)] = t.ap()
    nc.all_engine_barrier()

    with tile.TileContext(nc) as tc:
        with tc.tile_pool(name="p", bufs=3) as pool:
            for c in range(ROW_BLOCKS):
                for d in range(COL_BLOCKS):
                    rs = slice(c * P, (c + 1) * P)
                    cs = slice(d * FD, (d + 1) * FD)

                    tx = pool.tile([P, FD], F16, tag="x")
                    nc.sync.dma_start(tx[:], x[rs, cs])

                    tm = pool.tile([P, FD], F32, tag="m")
                    nc.vector._custom_dve(
                        poly_op, out=tm[:], in0=tx[:],
                        s0=PC0, s1=PC1, imm2=PC2)

                    tiv = pool.tile([P, FD], F32, tag="iv")
                    nc.scalar.activation(
                        tiv[:], tm[:], AF.Exp, bias=PBEXP, scale=PS)

                    to = pool.tile([P, FD], F16, tag="o")
                    nc.scalar.activation(to[:], tiv[:], AF.Ln, bias=EPS)

                    nc.sync.dma_start(y[rs, cs], to[:])

    nc.compile()
    return nc


def _get_nc():
    global _nc_cache
    if _nc_cache is None:
        _nc_cache = _build()
    return _nc_cache


def make_in_maps(kappa: np.ndarray):
    """Host-side marshalling: prescale + fp16 quantize, shard by row blocks."""
    t = (np.asarray(kappa, dtype=np.float32) * np.float32(ALPHA)).astype(
        np.float16)
    return [
        {"x": np.ascontiguousarray(t[i * SH_ROWS:(i + 1) * SH_ROWS])}
        for i in range(N_CORES)
    ]


def kernel(kappa: np.ndarray) -> np.ndarray:
    kappa = np.asarray(kappa)
    assert kappa.shape == (ROWS, COLS)
    nc = _get_nc()
    res = bass_utils.run_bass_kernel_spmd(
        nc, make_in_maps(kappa), core_ids=list(range(N_CORES)))
    out = np.concatenate(
        [res.results[i]["y"] for i in range(N_CORES)], axis=0)
    return out.astype(np.float32)


# revision 5
# speedup vs baseline: 3.2082x; 1.1241x over previous
"""Trainium2 Bass kernel for nn_LogBessel: out = log(I_31(kappa) + 1e-10).

Math: the reference's f(x) = ln(exp(ln I_31(x)) + eps) is approximated as

    f(x) ~= ln(exp(PS*M(t) + PBEXP) + eps),   t = ALPHA * x  (host prescale)
    M(t) = (((t + PC0)*t + PC1)*t + PC2)*t    (monic degree-4 Horner)

The Exp/Ln pair reproduces the reference's exp -> +eps -> log structure
exactly, so the small-x regime (output == log(1e-10)) matches by
construction, and the deg-4 polynomial only needs accuracy where
exp(g) >~ eps (x in ~[11, 50]).  Offline end-to-end minimax fit:
max |f_hat - f| ~= 0.092 with fp16 I/O -- ~8x under the harness gate.

Two compute paths share the load so neither engine is the bottleneck
(DVE ~26 us, ACT ~26 us per core, overlapped):

  path B (12288 of 16384 cols/core):
    DVE:  ONE custom-DVE op (6 ALU stages) evaluates M(t) per tile.
    ACT:  iv = Exp(PS*M + PBEXP);  f = Ln(iv + eps).
  path A (4096 cols/core, rows 384..512 of each shard):
    DVE:  same M op;  p = M*PS + PB  (stock tensor_scalar, 2x mode);
          f - C = max(p,0) + BG*relu(min(BB - p, BB + p))^2
          (second custom-DVE op: softplus approx, max err 0.035).
    host: adds C = ln(eps) to path-A rows after the upcast.

DMA: fp16 both directions (~8.4 MB/core).  Inputs as 4 row-block
mega-DMAs issued up front on SP; outputs per compute tile issued from
the otherwise-idle GpSimd queue so SP's issue stream never blocks on
compute.  First/last tiles are narrow to shorten ramp and drain.

Sharding: trivially data-parallel; 4096 rows split into 8 blocks of 512,
one per NeuronCore (same SPMD program, different data).
"""

import numpy as np

from concourse import bacc, mybir, tile
from concourse import bass_utils

F16 = mybir.dt.float16
F32 = mybir.dt.float32
AF = mybir.ActivationFunctionType
OP = mybir.AluOpType

N_CORES = 8
ROWS, COLS = 4096, 4096
SH_ROWS = ROWS // N_CORES          # 512 rows per core
P = 128                            # SBUF partitions
RB = SH_ROWS // P                  # 4 row blocks per core

# --- fitted constants (offline minimax fit; see module docstring) ---
ALPHA = 0.259912064                # host prescale: t = ALPHA * kappa
PC0 = -40.4114600                  # M = (((t+PC0)*t+PC1)*t+PC2)*t
PC1 = 636.127312
PC2 = -6194.10922
PS = -3.11125819e-3                # iv = exp(PS*M + PBEXP)
PBEXP = -65.7643952                # = b + ln(eps)
EPS = 1e-10
CLN = float(np.log(1e-10))         # ln(eps)
PB = PBEXP - CLN                   # p = PS*M + PB  (path A)
BG = 0.05060354                    # bump gain   (softplus approx)
BB = 3.60547846                    # bump half-width

# program-ordered tiles: (row_block, col0, col1, is_path_a)
TILES = [
    (0, 0, 512, False),
    (0, 512, 2560, False),
    (3, 0, 2048, True),
    (1, 0, 2048, False),
    (1, 2048, 4096, False),
    (3, 2048, 4096, True),
    (2, 0, 2048, False),
    (2, 2048, 4096, False),
    (0, 2560, 4096, False),
]
MEGA_ORDER = [0, 3, 1, 2]          # row-block DMA issue order

_nc_cache = None

_ACT_SET = "natural_log_exp_and_others"


def _force_single_act_set():
    """Make ln/exp resolvable only from natural_log_exp_and_others so
    walrus's per-function set assignment cannot ping-pong table loads."""
    import json, tempfile, os
    try:
        from neuronxcc.driver.jobs.support import FindActInfo
        from neuronxcc.driver.jobs import WalrusDriver as WD
    except ImportError:
        return
    if getattr(FindActInfo, "_logbessel_patched", False):
        return
    orig = FindActInfo.findActInfoFile

    def patched(package_dir, arch):
        path = orig(package_dir, arch)
        try:
            import shutil
            # table .bin blobs are resolved relative to the json, so clone
            # the whole pwp_bin dir and patch the json inside the clone
            dst = os.path.join(tempfile.gettempdir(), "pwp_single_set")
            if not os.path.isdir(dst):
                shutil.copytree(os.path.dirname(path), dst)
            d = json.load(open(path))
            for s in d.get("act_func_sets", []):
                if s.get("name") != _ACT_SET:
                    for fn in ("ln", "exp", "square"):
                        s.get("act", {}).pop(fn, None)
            out = os.path.join(dst, "act_info.json")
            with open(out, "w") as f:
                json.dump(d, f)
            return out
        except Exception:
            return path

    patched._logbessel_patched = True
    FindActInfo._logbessel_patched = True
    FindActInfo.findActInfoFile = patched
    WD.findActInfoFile = patched


_POLY_OP = "LOGBESSEL_P4_ANT"
_BUMP_OP = "SOFTPLUS_BUMP_ANT"


def _register_custom_ops():
    """Register the two custom DVE ops (each one streaming pass per tile):
      poly: out = (((in0 + s0)*in0 + s1)*in0 + imm2)*in0          (6 stages)
      bump: out = max(in0,0) + imm2*relu(min(s0 - in0, s1 + in0))^2 (8 stages)
    """
    import concourse.dve_ops as dve_ops_mod
    from concourse.dve_ops import DveOp
    from concourse.dve_spec import (
        Spec, Src0, C0, C1, C2, Zero, relu, sq, maxx, minn,
        lower as dve_lower,
    )
    from concourse.dve_uop import DveOpSpec

    def reg(name, spec):
        for op in dve_ops_mod.OPS:
            if op.name == name:
                return op
        row = max(dve_ops_mod._SUB_OPCODE_FOR_NAME.values()) + 1
        assert row < 0x20, "custom-DVE 5-bit row space exhausted"
        dve_ops_mod._SUB_OPCODE_FOR_NAME[name] = row
        shas = {}
        for ver in ("v3", "v4"):
            uops = dve_lower(spec, ver=ver)
            shas[ver] = DveOpSpec(
                name=name, opcode=row, uops=uops, rd1_en=False
            ).sha(ver)
        op = DveOp(name, spec, subdim=False, uops_sha=shas)
        dve_ops_mod.OPS.append(op)
        dve_ops_mod.CUSTOM_DVE_SPECS[name] = spec
        return op

    poly = reg(_POLY_OP, Spec(
        body=(((Src0 + C0) * Src0 + C1) * Src0 + C2) * Src0,
        reference=lambda in0, in1, s0, s1, imm2: (
            (((in0.astype(np.float32) + s0) * in0 + s1) * in0 + imm2) * in0
        ).astype(np.float32),
    ))
    bump = reg(_BUMP_OP, Spec(
        body=maxx(Src0, Zero)
        + sq(relu(minn(C0 - Src0, C1 + Src0))) * C2,
        reference=lambda in0, in1, s0, s1, imm2: (
            np.maximum(in0.astype(np.float32), 0.0)
            + imm2 * np.maximum(
                np.minimum(s0 - in0, s1 + in0), 0.0) ** 2
        ).astype(np.float32),
    ))
    return poly, bump


def _build():
    _force_single_act_set()
    poly_op, bump_op = _register_custom_ops()

    nc = bacc.Bacc("TRN2", target_bir_lowering=False, debug=False)
    x = nc.dram_tensor("x", [SH_ROWS, COLS], F16, kind="ExternalInput").ap()
    y = nc.dram_tensor("y", [SH_ROWS, COLS], F16, kind="ExternalOutput").ap()

    # activation() requires float biases to exist as [128,1] const SBUF
    # tensors; register ours the same way Bass.__init__ registers 0.0/1.0.
    for val in (PBEXP, EPS):
        t = nc.alloc_sbuf_tensor(f"const-f32-{val}", [128, 1], F32)
        nc.gpsimd.memset(t.ap(), val)
        nc.const_aps.aps[(F32, val)] = t.ap()
    nc.all_engine_barrier()

    with tile.TileContext(nc) as tc:
        with tc.tile_pool(name="pm", bufs=1) as mpool, \
             tc.tile_pool(name="p", bufs=3) as pool:
            # all 4 row-block input mega-DMAs issued up front on SP
            mega = {}
            for rb in MEGA_ORDER:
                mx = mpool.tile([P, COLS], F16, tag=f"mx{rb}")
                nc.sync.dma_start(mx[:], x[rb * P:(rb + 1) * P, :])
                mega[rb] = mx

            for rb, c0, c1, is_a in TILES:
                w = c1 - c0
                tx = mega[rb]

                tm = pool.tile([P, w], F32, tag=f"m{w}")
                nc.vector._custom_dve(
                    poly_op, out=tm[:], in0=tx[:, c0:c1],
                    s0=PC0, s1=PC1, imm2=PC2)

                to = pool.tile([P, w], F16, tag=f"o{w}")
                if is_a:
                    tp_ = pool.tile([P, w], F32, tag=f"p{w}")
                    nc.vector.tensor_scalar(
                        tp_[:], tm[:], PS, PB, op0=OP.mult, op1=OP.add)
                    nc.vector._custom_dve(
                        bump_op, out=to[:], in0=tp_[:],
                        s0=BB, s1=BB, imm2=BG)
                else:
                    tiv = pool.tile([P, w], F32, tag=f"iv{w}")
                    nc.scalar.activation(
                        tiv[:], tm[:], AF.Exp, bias=PBEXP, scale=PS)
                    nc.scalar.activation(to[:], tiv[:], AF.Ln, bias=EPS)

                nc.gpsimd.dma_start(y[rb * P:(rb + 1) * P, c0:c1], to[:])

    nc.compile()
    return nc


def _get_nc():
    global _nc_cache
    if _nc_cache is None:
        _nc_cache = _build()
    return _nc_cache


def make_in_maps(kappa: np.ndarray):
    """Host-side marshalling: prescale + fp16 quantize, shard by row blocks."""
    t = (np.asarray(kappa, dtype=np.float32) * np.float32(ALPHA)).astype(
        np.float16)
    return [
        {"x": np.ascontiguousarray(t[i * SH_ROWS:(i + 1) * SH_ROWS])}
        for i in range(N_CORES)
    ]


def kernel(kappa: np.ndarray) -> np.ndarray:
    kappa = np.asarray(kappa)
    assert kappa.shape == (ROWS, COLS)
    nc = _get_nc()
    res = bass_utils.run_bass_kernel_spmd(
        nc, make_in_maps(kappa), core_ids=list(range(N_CORES)))
    out = np.concatenate(
        [res.results[i]["y"] for i in range(N_CORES)], axis=0)
    out = out.astype(np.float32)
    # path-A tiles (row block 3 of each shard) return f - ln(eps)
    for i in range(N_CORES):
        out[i * SH_ROWS + 3 * P:(i + 1) * SH_ROWS] += np.float32(CLN)
    return out
